# revision 32
# baseline (speedup 1.0000x reference)
"""DoubleAttention (Performer global heads + local windowed heads) on 8
Trainium2 NeuronCores via Bass/Tile SPMD.

I/O-lean sharding (v2). 8 cores = 2 batch-groups of 4: cores 0-3 own
batch 0, cores 4-7 batch 1. Within a group, cores rank r=0..3 each ship
ONE natural-layout quarter of x ([2048,512], a zero-copy row view), so
H2D for x is exactly |x|. On device each core transposes its quarter
(PE transposes) and the group AllGathers the feature-major x into DRAM.
Rotary cos/sin tables ship as per-core quarters and are AllGathered the
same way. Cores 0,1,2 (4,5,6) compute one pair of the 6 global
Performer heads over all 8192 tokens; cores 3,7 compute both local
windowed heads. Each core adds bo/4 (rank-1 matmul) to its partial
y = attn_out @ Wo[slice,:]; a ReduceScatter(add) over the group then
hands each core the FINAL y rows for its token quarter, so D2H is
exactly |y| and the host does no arithmetic at all.

Exact math restructurings vs the reference (fp-rounding-level equal):
 - Performer `ratio` cancels between numerator and denominator.
 - dd - diag fused into ONE matmul: contract [qT*norm ; qT^2*0.5norm^2]
   (128 rows) against [projT ; -ones].
 - per-query max only affects the eps floor: exp(z-m)+eps is a per-token
   scale (cancels) times exp(z) + eps*e^{m}; the latter enters num/den
   as an appended rank-1 term.
 - k-side global max applied post-hoc: ctx = e^{-m_k}*sum(exp(z_k)[v|1])
   + eps*[vsum|N].
 - local attention: softmax max-subtraction dropped (dots are O(5), exp
   safe in fp32; softmax is shift-invariant). P = exp(dots/8) computed
   KEY-major; [v|1] folds the row-sum into the same A@V matmul.

All heavy matmuls run as float32r (~2e-4 rel err, same order as this
PE's fp32 mode, at 4x throughput). Engines cannot shift partitions
(lane-locked), so the few cross-partition moves go through SBUF->SBUF
DMAs or PE transposes.
"""
import numpy as np
from contextlib import ExitStack

import concourse.bass as bass
import concourse.mybir as mybir
import concourse.tile as tile
from concourse.masks import make_identity

F32 = mybir.dt.float32
F32R = mybir.dt.float32r
BF16 = mybir.dt.bfloat16
AF = mybir.ActivationFunctionType
ALU = mybir.AluOpType
AX = mybir.AxisListType

DM = 512
DH = 64
NF = 256
WIN = 256
EPS = 1e-4
NORM = DH ** -0.25
SQRT_C2 = (0.5 * NORM * NORM) ** 0.5    # Square(x*s) = x^2 * 0.5norm^2
GROUPS = [[0, 1, 2, 3], [4, 5, 6, 7]]
PAIRS = [[0, 4], [1, 5], [2, 6], [3, 7]]

# static-input blob row offsets (512-f32-wide rows). Cores c and c+4 need
# IDENTICAL static data (weights by head-pair c%4, rotary by rank c%4), so
# each ships only half the blob and a pair AllGather reassembles it.
R_WQ, R_WK, R_WV, R_WO, R_PROJ, R_BO, R_COS = 0, 128, 256, 384, 512, 544, 545


def _blob_rows(NQ):
    ncos = DH * NQ // 512
    r_sin = R_COS + ncos
    r_end = r_sin + ncos
    r_tot = r_end + (r_end & 1)
    return ncos, r_sin, r_end, r_tot

# ---------------------------------------------------------------------------
# walrus wait legalizer: this toolchain's walrus accepts only ONE sync wait
# per instruction; Tile attaches several. Split extras onto NoOps.
# ---------------------------------------------------------------------------
_WNOP = [0]


def _nop(engine, debug, waits=(), updates=()):
    _WNOP[0] += 1
    return {
        "name": f"WNOP-{_WNOP[0]}",
        "opcode": "NoOp",
        "engine": engine,
        "ins": [],
        "outs": [],
        "debug": debug,
        "sync_info": {"on_update": list(updates), "on_wait": list(waits)},
    }


def _legalize_bir_waits(bir_bytes: bytes) -> bytes:
    """Engine instruction structs accept ONE wait and ONE sem-inc(+1)
    update. Tile emits several waits per instruction and (at If-arm clock
    merges) big sem-add-imm updates. Split extras onto NoOps on the same
    queue (engines retire in order, so ordering semantics hold)."""
    import orjson
    d = orjson.loads(bir_bytes)
    for fn in d["functions"]:
        for bb in fn["blocks"]:
            out = []
            for inst in bb["instructions"]:
                op = inst.get("opcode", "")
                si = inst.get("sync_info")
                if si is None or "Branch" in op:
                    out.append(inst)
                    continue
                dbg = inst.get("debug")
                eng = inst["engine"]
                if "DMA" not in op.upper() and op != "ISA":
                    # A big add-imm comes from If-arm clock alignment: v-1
                    # virtual ticks (guarding no data) + this instruction's
                    # own completion. Emit the padding as +1 NoOps BEFORE
                    # the instruction (a trailing NoOp would fire at issue,
                    # before the writes drain) and keep +1 on it.
                    ups = si.get("on_update") or []
                    new_ups = []
                    for u in ups:
                        if (u.get("sync_type") == "semaphore"
                                and u.get("update_mode") in ("sem-inc",
                                                             "sem-add-imm")
                                and int(u.get("update_value", 1)) > 1):
                            v = int(u["update_value"])
                            out.append(_nop(eng, dbg, updates=[
                                dict(u, update_mode="sem-add-imm",
                                     update_value=v - 1)]))
                            new_ups.append(dict(u, update_mode="sem-inc",
                                                update_value=1))
                        else:
                            new_ups.append(u)
                    si["on_update"] = new_ups
                waits = si.get("on_wait") or []
                if len(waits) > 1:
                    for w in waits[:-1]:
                        out.append(_nop(eng, dbg, waits=[w]))
                    si["on_wait"] = [waits[-1]]
                out.append(inst)
            bb["instructions"] = out
    return orjson.dumps(d)


def _install_legalizer():
    import concourse.bass2jax as b2j
    if getattr(b2j, "_wait_legalizer_installed", False):
        return
    orig = b2j.compile_bir_kernel

    def patched(ant_bir_str, *args, **kwargs):
        return orig(_legalize_bir_waits(ant_bir_str), *args, **kwargs)

    b2j.compile_bir_kernel = patched
    b2j._wait_legalizer_installed = True


# ---------------------------------------------------------------------------
# program builder
# ---------------------------------------------------------------------------

class _Env:
    pass


def build_program(n_tok: int) -> bass.Bass:
    TT = 512
    NST = n_tok // TT
    NC = n_tok // 128
    NW = n_tok // WIN
    NQ = n_tok // 4          # tokens per core quarter
    TPC = NQ // TT           # 512-token tiles per quarter/chunk

    nc = bass.Bass(num_devices=8)
    e = _Env()
    e.n_tok, e.TT, e.NST, e.NC, e.NW, e.NQ, e.TPC = (
        n_tok, TT, NST, NC, NW, NQ, TPC)
    NCOS, R_SIN, R_END, R_TOT = _blob_rows(NQ)
    e.xq = nc.declare_dram_parameter("xq", [NQ, DM], F32, isOutput=False)
    e.blob_h = nc.declare_dram_parameter("blob_h", [R_TOT // 2, DM], F32,
                                         isOutput=False)
    e.y = nc.declare_dram_parameter("y", [NQ, DM], F32, isOutput=True)

    with ExitStack() as ctx:
        tc = ctx.enter_context(tile.TileContext(nc))

        # ---- DRAM scratch (collective bounces + gathered views) ----
        # x gather and y reduce-scatter are split 4 ways and pipelined:
        # sub-gather s covers 512-token sub-blocks s of every rank quarter
        # (consumed s-major), output group j covers final rows
        # r*NQ + j*512.. of every rank r (produced j-major).
        dram = ctx.enter_context(tc.tile_pool(name="dram", bufs=1, space="DRAM"))
        e.xtq_b = dram.tile([4, 128, 4, TT], F32)    # my transposed quarter
        e.xg = dram.tile([4, 4, 128, 4, TT], F32)    # [s, rank, p, c, t]
        e.blob_b = dram.tile([R_TOT // 2, DM], F32)  # pair-AG bounce
        e.blob = dram.tile([R_TOT, DM], F32)         # pair-gathered statics
        e.cosg = dram.tile([4, DH, NQ], F32)
        e.sing = dram.tile([4, DH, NQ], F32)
        e.yacc = dram.tile([4, 4, TT, DM], F32)      # [j, rank, u, d] partials
        e.yrs = dram.tile([NQ, DM], F32)             # reduce-scattered y rows

        # ---- reassemble the pair-shared statics, then load them ----
        nc.gpsimd.dma_start(e.blob_b[:], e.blob_h[:])
        nc.gpsimd.collective_compute(
            "AllGather", ALU.bypass, replica_groups=PAIRS,
            ins=[e.blob_b.opt()], outs=[e.blob.opt()])

        # ---- shared preamble ----
        pre = ctx.enter_context(tc.tile_pool(name="pre", bufs=1))
        e.ident = pre.tile([128, 128], F32)
        make_identity(nc, e.ident[:])

        e.wq_r = pre.tile([128, 4, 128], F32R)
        e.wk_r = pre.tile([128, 4, 128], F32R)
        e.wv_r = pre.tile([128, 4, 128], F32R)
        for w_sb, r0 in ((e.wq_r, R_WQ), (e.wk_r, R_WK), (e.wv_r, R_WV)):
            nc.sync.dma_start(
                w_sb[:], e.blob[r0:r0 + 128, :].bitcast(F32R).rearrange(
                    "p (c f) -> p c f", c=4))
        e.wo_r = pre.tile([128, DM], F32R)
        nc.sync.dma_start(e.wo_r[:], e.blob[R_WO:R_WO + 128, :].bitcast(F32R))
        e.bo4_row = pre.tile([1, DM], F32)
        nc.sync.dma_start(e.bo4_row[:], e.blob[R_BO:R_BO + 1, :])
        e.ones_1x128 = pre.tile([1, 128], F32)
        nc.gpsimd.memset(e.ones_1x128[:], 1.0)

        pn_f = pre.tile([128, NF + 4], F32)
        nc.sync.dma_start(pn_f[0:DH, 0:NF],
                          e.blob[R_PROJ:R_PROJ + 32, :].rearrange(
                              "a (b c) -> (a b) c", b=2))
        nc.gpsimd.memset(pn_f[DH:128, 0:NF], -1.0)
        # col 256 extracts diag (sum of the squared half); 257-259 pad (f32r
        # moving free dim must be a multiple of 4)
        nc.gpsimd.memset(pn_f[0:DH, NF:NF + 4], 0.0)
        nc.gpsimd.memset(pn_f[DH:128, NF:NF + 4], 0.0)
        nc.gpsimd.memset(pn_f[DH:128, NF:NF + 1], 1.0)
        e.projnegP_r = pre.tile([128, NF + 4], F32R)
        nc.vector.tensor_copy(e.projnegP_r[:], pn_f[:])
        e.projneg_r = e.projnegP_r[:, 0:NF]

        ones_f = pre.tile([128, 1], F32)
        nc.gpsimd.memset(ones_f[:], 1.0)
        e.ones_f = ones_f
        e.ones_col_r = pre.tile([128, 1], F32R)
        nc.vector.tensor_copy(e.ones_col_r[:], ones_f[:])
        e.ones_row65 = pre.tile([1, 65], F32)
        nc.gpsimd.memset(e.ones_row65[:], 1.0)
        e.lneps = pre.tile([128, 1], F32)
        nc.gpsimd.memset(e.lneps[:], float(np.log(EPS)))

        # ---- shared residents (used by BOTH branches; SBUF is core-local) ----
        res = ctx.enter_context(tc.tile_pool(name="res", bufs=1))
        e.R1 = res.tile([128, n_tok], F32R)        # global: Aq head0 / local: qTr
        e.R2 = res.tile([128, n_tok], F32R)        # global: Aq head1 / local: kTr
        e.R3 = res.tile([128, NC, 130], F32R)      # v token-major [v0|1|v1|1]
        e.mk_buf = res.tile([128, 2, NC], F32)
        e.vsum_buf = res.tile([128, max(NST, 2)], F32)
        e.ctx_fm = res.tile([128, 2, 2, 68], F32R)  # [p, mchunk, head, col] (68: f32r moving needs %4)
        e.s_row = res.tile([1, 2, 68], F32R)

        # init the ones columns of R3 once (cols 64 and 129 of each chunk)
        for kc in range(NC):
            nc.vector.tensor_copy(e.R3[:, kc, 64:65], e.ones_col_r[:])
            nc.vector.tensor_copy(e.R3[:, kc, 129:130], e.ones_col_r[:])

        # ---- transpose own x quarter and gather the group's xT ----
        # scoped pools: this phase's SBUF/PSUM is released before the
        # branch working pools open (SBUF is within 8KB of full).
        with tc.tile_pool(name="xp", bufs=2) as xp, \
                tc.tile_pool(name="psX", bufs=2, space="PSUM") as psX:
            nc.gpsimd.collective_compute(
                "AllGather", ALU.bypass, replica_groups=GROUPS,
                ins=[e.blob[R_COS:R_COS + NCOS, :].opt()], outs=[e.cosg.opt()])
            nc.gpsimd.collective_compute(
                "AllGather", ALU.bypass, replica_groups=GROUPS,
                ins=[e.blob[R_SIN:R_SIN + NCOS, :].opt()], outs=[e.sing.opt()])
            for s in range(4):
                for ii in range(4):
                    i = s * 4 + ii
                    xn = xp.tile([128, DM], F32, tag="xn")
                    nc.sync.dma_start(xn[:], e.xq[i * 128:(i + 1) * 128, :])
                    xtq = xp.tile([128, 4, 128], F32, tag="xtq")
                    for c in range(4):
                        tr_ps = psX.tile([128, 128], F32, tag="smx")
                        nc.tensor.transpose(tr_ps[:],
                                            xn[:, c * 128:(c + 1) * 128],
                                            e.ident[:])
                        nc.vector.tensor_copy(xtq[:, c, :], tr_ps[:])
                    nc.sync.dma_start(
                        e.xtq_b[s, :, :, ii * 128:(ii + 1) * 128], xtq[:])
                nc.gpsimd.collective_compute(
                    "AllGather", ALU.bypass, replica_groups=GROUPS,
                    ins=[e.xtq_b[s].opt()], outs=[e.xg[s].opt()])

        # ---- shared pools (tags shared across branches to bound SBUF) ----
        e.ld = ctx.enter_context(tc.tile_pool(name="ld", bufs=2))
        e.wk3 = ctx.enter_context(tc.tile_pool(name="wk3", bufs=3))
        e.wk2 = ctx.enter_context(tc.tile_pool(name="wk2", bufs=2))
        e.psProj = ctx.enter_context(tc.tile_pool(name="psProj", bufs=3, space="PSUM"))
        e.psAcc = ctx.enter_context(tc.tile_pool(name="psAcc", bufs=2, space="PSUM"))
        e.psSm = ctx.enter_context(tc.tile_pool(name="psSm", bufs=3, space="PSUM"))

        pid = nc.partition_id()
        is_global = (pid & 3) < 3
        with tc.If(is_global) as cmp:
            _global_phase1(nc, tc, e)
        with cmp.Else():
            _local_phase1(nc, tc, e)

        # ---- output groups: compute j while reduce-scattering j-1 ----
        for j in range(4):
            with tc.If(is_global) as cmpj:
                _global_out_j(nc, tc, e, j)
            with cmpj.Else():
                _local_out_j(nc, tc, e, j)
            nc.gpsimd.collective_compute(
                "ReduceScatter", ALU.add, replica_groups=GROUPS,
                ins=[e.yacc[j].opt()], outs=[e.yrs[j * TT:(j + 1) * TT, :].opt()])
            nc.gpsimd.dma_start(e.y[j * TT:(j + 1) * TT, :],
                                e.yrs[j * TT:(j + 1) * TT, :])

    return nc


def _tr(nc, e, out_ap, in_ap):
    k = in_ap.shape[0]
    nc.tensor.transpose(out_ap, in_ap, e.ident[0:k, 0:k])


def _load_xt(nc, e, t):
    s, r = t % e.TPC, t // e.TPC
    xt = e.ld.tile([128, 4, e.TT], F32R, tag="xt")
    nc.sync.dma_start(xt[:], e.xg[s, r].bitcast(F32R))
    return xt


def _tile_order(e):
    """s-major: tiles using sub-gather s come before any using s+1, so
    compute on s overlaps the AllGather of s+1."""
    return [r * e.TPC + s for s in range(e.TPC) for r in range(4)]


def _project(nc, e, xt, w_r):
    """q/k/v projection into PSUM [128 = 2 heads x 64, TT]."""
    pp = e.psProj.tile([128, e.TT], F32, tag="proj")
    for c in range(4):
        nc.tensor.matmul(pp[:], w_r[:, c, :], xt[:, c, :],
                         start=(c == 0), stop=(c == 3))
    return pp


def _v_tokmajor(nc, e, t, v_ps):
    """v [128, TT] PSUM d-major -> R3 chunks [tok128, v0|1|v1|1]."""
    v_sb = e.wk2.tile([128, e.TT], F32, tag="vsb")
    nc.scalar.activation(v_sb[:], v_ps[:], AF.Identity,
                         accum_out=e.vsum_buf[:, t:t + 1])
    for su in range(4):
        kc = t * 4 + su
        vtr_ps = e.psSm.tile([128, 128], F32, tag="sm")
        _tr(nc, e, vtr_ps[:], v_sb[:, su * 128:(su + 1) * 128])
        nc.vector.tensor_copy(e.R3[:, kc, 0:64], vtr_ps[:, 0:64])
        nc.vector.tensor_copy(e.R3[:, kc, 65:129], vtr_ps[:, 64:128])


def _aug_assemble(nc, e, p_ps, dest0, dest1, ts, tag):
    """[2-head packed PSUM [128,TT]] -> per-head augmented [n*NORM ; n^2*c2]
    written into dest0/dest1 [128, ts]. Lane engines can't shift partitions,
    so the cross-half moves are SBUF->SBUF DMAs."""
    qn = e.wk2.tile([128, e.TT], F32R, tag=f"{tag}n")
    nc.scalar.mul(qn[:], p_ps[:], NORM)
    sq = e.wk2.tile([128, e.TT], F32R, tag=f"{tag}s")
    nc.scalar.activation(sq[:], p_ps[:], AF.Square, scale=SQRT_C2)
    nc.vector.tensor_copy(dest0[0:64, ts], qn[0:64, :])
    nc.sync.dma_start(dest0[64:128, ts], sq[0:64, :])
    nc.sync.dma_start(dest1[0:64, ts], qn[64:128, :])
    nc.vector.tensor_copy(dest1[64:128, ts], sq[64:128, :])


def _global_phase1(nc, tc, e):
    NST, NC, TT, n_tok = e.NST, e.NC, e.TT, e.n_tok
    Aq = [e.R1, e.R2]

    # ---------------- phase G1: k/v side + Aq build ----------------
    ctx_ps = []
    for h in range(2):
        acc_t = e.psAcc.tile([65, NF], F32, tag="acc", name=f"acc{h}")
        ctx_ps.append(acc_t)
    order = _tile_order(e)
    for it, t in enumerate(order):
        ts = slice(t * TT, (t + 1) * TT)
        xt = _load_xt(nc, e, t)

        q_ps = _project(nc, e, xt, e.wq_r)
        _aug_assemble(nc, e, q_ps, Aq[0], Aq[1], ts, "q")

        k_ps = _project(nc, e, xt, e.wk_r)
        ak0 = e.wk2.tile([128, TT], F32R, tag="ak0")
        ak1 = e.wk2.tile([128, TT], F32R, tag="ak1")
        _aug_assemble(nc, e, k_ps, ak0, ak1, slice(0, TT), "k")
        ak = [ak0, ak1]

        v_ps = _project(nc, e, xt, e.wv_r)
        _v_tokmajor(nc, e, t, v_ps)

        for su in range(4):
            kc = t * 4 + su
            ss = slice(su * 128, (su + 1) * 128)
            for h in range(2):
                zk_ps = e.psSm.tile([128, NF + 4], F32, tag="sm")
                nc.tensor.matmul(zk_ps[:], ak[h][:, ss], e.projnegP_r[:],
                                 start=True, stop=True)
                # reference maxes are over dd = z + diag (diag in col 256)
                zmax = e.wk3.tile([128, 1], F32, tag="zmax")
                nc.vector.reduce_max(zmax[:], zk_ps[:, 0:NF], axis=AX.X)
                nc.vector.tensor_tensor(e.mk_buf[:, h, kc:kc + 1], zmax[:],
                                        zk_ps[:, NF:NF + 1], ALU.add)
                kp = e.wk3.tile([128, NF], F32R, tag="kp")
                nc.scalar.activation(kp[:], zk_ps[:, 0:NF], AF.Exp)
                nc.tensor.matmul(ctx_ps[h][:], e.R3[:, kc, h * 65:(h + 1) * 65],
                                 kp[:], start=(it == 0 and su == 0),
                                 stop=(it == NST - 1 and su == 3))

    # ---- k-side fixups ----
    vsum = e.wk2.tile([128, 1], F32, tag="vsum")
    nc.vector.reduce_sum(vsum[:], e.vsum_buf[:, 0:NST], axis=AX.X)
    vst_ps = e.psSm.tile([1, 128], F32, tag="sm")
    _tr(nc, e, vst_ps[:], vsum[:])
    vsumT = e.wk2.tile([1, 128], F32, tag="vsumT", bufs=1)
    nc.vector.tensor_copy(vsumT[:], vst_ps[:])

    for h in range(2):
        mk_red = e.wk2.tile([128, 1], F32, tag="mkred")
        nc.vector.reduce_max(mk_red[:], e.mk_buf[:, h, :], axis=AX.X)
        mkt_ps = e.psSm.tile([1, 128], F32, tag="sm")
        _tr(nc, e, mkt_ps[:], mk_red[:])
        mkt = e.wk2.tile([1, 128], F32, tag="mkt", bufs=1)
        nc.vector.tensor_copy(mkt[:], mkt_ps[:])
        mk_sc = e.wk2.tile([1, 1], F32, tag="mksc")
        nc.vector.reduce_max(mk_sc[:], mkt[:], axis=AX.X)
        f_sc = e.wk2.tile([1, 1], F32, tag="fsc")
        nc.scalar.activation(f_sc[:], mk_sc[:], AF.Exp, scale=-1.0)
        fb_ps = e.psSm.tile([65, 1], F32, tag="sm")
        nc.tensor.matmul(fb_ps[:], e.ones_row65[:], f_sc[:], start=True, stop=True)
        fb = e.wk2.tile([65, 1], F32, tag="fb")
        nc.vector.tensor_copy(fb[:], fb_ps[:])

        ev_row = e.wk2.tile([1, 65], F32, tag="evrow", bufs=1)
        nc.scalar.mul(ev_row[:, 0:64], vsumT[:, h * DH:(h + 1) * DH], EPS)
        nc.gpsimd.memset(ev_row[:, 64:65], EPS * n_tok)
        ev_ps = e.psSm.tile([65, 1], F32, tag="sm")
        _tr(nc, e, ev_ps[:], ev_row[:])
        epsv = e.wk2.tile([65, 1], F32, tag="epsv", bufs=1)
        nc.vector.tensor_copy(epsv[:], ev_ps[:])

        ctxT = e.wk2.tile([65, NF], F32, tag="ctxT", bufs=1)
        nc.vector.tensor_scalar(ctxT[:], ctx_ps[h][:], fb[:], epsv[:],
                                ALU.mult, ALU.add)
        for c in range(2):
            cf_ps = e.psSm.tile([128, 65], F32, tag="sm")
            _tr(nc, e, cf_ps[:], ctxT[:, c * 128:(c + 1) * 128])
            nc.vector.tensor_copy(e.ctx_fm[:, c, h, 0:65], cf_ps[:])
            nc.vector.tensor_copy(e.ctx_fm[:, c, h, 65:68], cf_ps[:, 0:3])
        sr_ps = e.psSm.tile([1, 65], F32, tag="sm")
        for c in range(2):
            nc.tensor.matmul(sr_ps[:], e.ones_f[:],
                             e.ctx_fm[:, c, h, 0:65].bitcast(F32),
                             start=(c == 0), stop=(c == 1))
        nc.vector.tensor_copy(e.s_row[:, h, 0:65], sr_ps[:])
        nc.vector.tensor_copy(e.s_row[:, h, 65:68], sr_ps[:, 0:3])


def _global_out_j(nc, tc, e, j):
    # ---------------- phase G2: q side, output group j ----------------
    NST, TT = e.NST, e.TT
    Aq = [e.R1, e.R2]
    for r in range(4):
        t = r * e.TPC + j
        ts = slice(t * TT, (t + 1) * TT)
        qp = [[None, None], [None, None]]
        ert = [None, None]
        for h in range(2):
            for c in range(2):
                zf_ps = e.psProj.tile([128, TT], F32, tag="proj")
                nc.tensor.matmul(zf_ps[:], e.projneg_r[:, c * 128:(c + 1) * 128],
                                 Aq[h][:, ts], start=True, stop=True)
                qp_c = e.wk2.tile([128, TT], F32R, tag=f"qp{h}{c}")
                nc.scalar.activation(qp_c[:], zf_ps[:], AF.Exp)
                qp[h][c] = qp_c
            ert_h = []
            for su in range(4):
                zt_ps = e.psSm.tile([128, NF + 4], F32, tag="sm")
                nc.tensor.matmul(
                    zt_ps[:], Aq[h][:, t * TT + su * 128: t * TT + (su + 1) * 128],
                    e.projnegP_r[:], start=True, stop=True)
                zmax = e.wk3.tile([128, 1], F32, tag="zmax")
                nc.vector.reduce_max(zmax[:], zt_ps[:, 0:NF], axis=AX.X)
                mq = e.wk3.tile([128, 1], F32, tag="mq")
                nc.vector.tensor_tensor(mq[:], zmax[:], zt_ps[:, NF:NF + 1], ALU.add)
                er = e.wk3.tile([128, 1], F32, tag="er")
                nc.scalar.activation(er[:], mq[:], AF.Exp, bias=e.lneps[:])
                ert_ps = e.psSm.tile([1, 128], F32, tag="sm")
                _tr(nc, e, ert_ps[:], er[:])
                ert_su = e.wk3.tile([1, 128], F32R, tag=f"ert{h}", name=f"ert{h}_{su}")
                nc.vector.tensor_copy(ert_su[:], ert_ps[:])
                ert_h.append(ert_su)
            ert[h] = ert_h

        for su in range(4):
            ss = slice(su * 128, (su + 1) * 128)
            row0 = t * TT + su * 128
            ao = e.wk3.tile([128, 128], F32, tag="ao")
            for h in range(2):
                nd_ps = e.psSm.tile([128, 68], F32, tag="sm")
                nc.tensor.matmul(nd_ps[:], qp[h][0][:, ss], e.ctx_fm[:, 0, h, :],
                                 start=True, stop=False)
                nc.tensor.matmul(nd_ps[:], qp[h][1][:, ss], e.ctx_fm[:, 1, h, :],
                                 start=False, stop=False)
                nc.tensor.matmul(nd_ps[:], ert[h][su][:], e.s_row[:, h, :],
                                 start=False, stop=True)
                rec = e.wk3.tile([128, 1], F32, tag="rec")
                nc.vector.reciprocal(rec[:], nd_ps[:, 64:65])
                nc.vector.tensor_scalar_mul(ao[:, h * DH:(h + 1) * DH],
                                            nd_ps[:, 0:64], rec[:])
            _project_out(nc, e, ao, row0)


def _project_out(nc, e, ao, row0):
    """attn-out token-major [128,128] -> transpose -> yacc rows via Wo slice
    (+ bo/4 as a rank-1 matmul; the 4 group partials sum to + bo)."""
    aoT_ps = e.psSm.tile([128, 128], F32, tag="sm")
    _tr(nc, e, aoT_ps[:], ao[:])
    aoT = e.wk3.tile([128, 128], F32R, tag="aoTs")
    nc.vector.tensor_copy(aoT[:], aoT_ps[:])
    y_ps = e.psProj.tile([128, DM], F32, tag="proj")
    nc.tensor.matmul(y_ps[:], aoT[:], e.wo_r[:], start=True, stop=False)
    nc.tensor.matmul(y_ps[:], e.ones_1x128[:], e.bo4_row[:],
                     start=False, stop=True)
    y_sb = e.wk2.tile([128, DM], F32, tag="ysb")
    nc.scalar.copy(y_sb[:], y_ps[:])
    r, rem = divmod(row0, e.NQ)
    j, u = divmod(rem, e.TT)
    nc.sync.dma_start(e.yacc[j, r, u:u + 128, :], y_sb[:])


def _local_phase1(nc, tc, e):
    NST, NC, TT, NW, TPC = e.NST, e.NC, e.TT, e.NW, e.TPC
    qTr, kTr = e.R1, e.R2

    # ---------------- phase L1: projections + rotary ----------------
    for t in _tile_order(e):
        ts = slice(t * TT, (t + 1) * TT)
        chunk, off = t // TPC, (t % TPC) * TT
        xt = _load_xt(nc, e, t)
        cos2 = e.ld.tile([128, TT], F32, tag="cos2")
        nc.sync.dma_start(cos2[0:DH, :], e.cosg[chunk, :, off:off + TT])
        nc.sync.dma_start(cos2[DH:128, :], cos2[0:DH, :])
        sin2 = e.ld.tile([128, TT], F32, tag="sin2")
        nc.sync.dma_start(sin2[0:DH, :], e.sing[chunk, :, off:off + TT])
        nc.sync.dma_start(sin2[DH:128, :], sin2[0:DH, :])

        for w_r, dest in ((e.wq_r, qTr), (e.wk_r, kTr)):
            pp = _project(nc, e, xt, w_r)
            p_sb = e.wk2.tile([128, TT], F32, tag="qn")
            nc.vector.tensor_copy(p_sb[:], pp[:])
            p_sw = e.wk2.tile([128, TT], F32, tag="qs")
            for h in range(2):
                o = h * DH
                nc.sync.dma_start(p_sw[o:o + 32, :], p_sb[o + 32:o + 64, :])
                nc.sync.dma_start(p_sw[o + 32:o + 64, :], p_sb[o:o + 32, :])
            t1 = e.wk2.tile([128, TT], F32, tag="kn")
            nc.vector.tensor_tensor(t1[:], p_sb[:], cos2[:], ALU.mult)
            t2 = e.wk2.tile([128, TT], F32, tag="ks")
            nc.vector.tensor_tensor(t2[:], p_sw[:], sin2[:], ALU.mult)
            nc.vector.tensor_tensor(dest[:, ts], t1[:], t2[:], ALU.add)

        v_ps = _project(nc, e, xt, e.wv_r)
        _v_tokmajor(nc, e, t, v_ps)


def _local_out_j(nc, tc, e, j):
    # ---------------- phase L2: windowed attention, output group j ----
    # Windows processed in PAIRS: adjacent windows share k-chunks, so one
    # [128, 512] dots matmul + one exp covers both windows per k-chunk
    # (halves the ACT op count and the dots matmul count).
    NC, NW, WINp = e.NC, e.NW, WIN
    qTr, kTr = e.R1, e.R2
    for r in range(4):
        wp = r * e.TPC + j
        wA, wB = 2 * wp, 2 * wp + 1
        qs = slice(wA * WIN, (wB + 1) * WIN)          # 512 queries
        cA0, cA1 = max(0, 2 * wA - 2), min(NC - 1, 2 * wA + 3)
        cB0, cB1 = max(0, 2 * wB - 2), min(NC - 1, 2 * wB + 3)
        olT_all = {}
        for h in range(2):
            hs = slice(h * DH, (h + 1) * DH)
            olA = e.psAcc.tile([65, WIN], F32, tag="acc", name=f"olA_{h}")
            olB = e.psAcc.tile([65, WIN], F32, tag="acc", name=f"olB_{h}")
            for kc in range(cA0, cB1 + 1):
                dk_ps = e.psProj.tile([128, 2 * WIN], F32, tag="proj")
                nc.tensor.matmul(dk_ps[:], kTr[hs, kc * 128:(kc + 1) * 128],
                                 qTr[hs, qs], start=True, stop=True)
                P = e.wk3.tile([128, 2 * WIN], F32R, tag="P2")
                nc.scalar.activation(P[:], dk_ps[:], AF.Exp, scale=0.125)
                if cA0 <= kc <= cA1:
                    nc.tensor.matmul(olA[:], e.R3[:, kc, h * 65:(h + 1) * 65],
                                     P[:, 0:WIN], start=(kc == cA0),
                                     stop=(kc == cA1))
                if cB0 <= kc <= cB1:
                    nc.tensor.matmul(olB[:], e.R3[:, kc, h * 65:(h + 1) * 65],
                                     P[:, WIN:2 * WIN], start=(kc == cB0),
                                     stop=(kc == cB1))
            for w, olp in ((wA, olA), (wB, olB)):
                olT_h = e.wk3.tile([65, WIN], F32, tag="olT", bufs=4,
                                   name=f"olT{w}_{h}")
                nc.vector.tensor_copy(olT_h[:], olp[:])
                olT_all[(h, w)] = olT_h
        for w in (wA, wB):
            olT = [olT_all[(0, w)], olT_all[(1, w)]]
            for su in range(2):
                row0 = w * WIN + su * 128
                ao = e.wk3.tile([128, 128], F32, tag="ao")
                for h in range(2):
                    tr_ps = e.psSm.tile([128, 65], F32, tag="sm")
                    _tr(nc, e, tr_ps[:], olT[h][:, su * 128:(su + 1) * 128])
                    rec = e.wk3.tile([128, 1], F32, tag="rec")
                    nc.vector.reciprocal(rec[:], tr_ps[:, 64:65])
                    nc.vector.tensor_scalar_mul(ao[:, h * DH:(h + 1) * DH],
                                                tr_ps[:, 0:64], rec[:])
                _project_out(nc, e, ao, row0)


# ---------------------------------------------------------------------------
# host wrapper: cached jitted shard_map runner, minimal copies
# ---------------------------------------------------------------------------
_RUNNER_CACHE = {}
_TABLE_CACHE = {}


def _rotary_tables(n_tok: int):
    if n_tok not in _TABLE_CACHE:
        inv_freq = 1.0 / (10000.0 ** (np.arange(0, DH, 2, dtype=np.float32) / DH))
        t = np.arange(n_tok, dtype=np.float32)
        freqs = t[:, None] * inv_freq[None, :]
        freqs = np.concatenate([freqs, freqs], axis=-1)
        cos = np.ascontiguousarray(np.cos(freqs).T.astype(np.float32))
        sin = np.sin(freqs).T.astype(np.float32)
        sinN = np.ascontiguousarray(
            np.concatenate([-sin[0:32], sin[32:64]], axis=0))
        _TABLE_CACHE[n_tok] = (cos, sinN)
    return _TABLE_CACHE[n_tok]


def _get_runner(n_tok: int):
    if n_tok in _RUNNER_CACHE:
        return _RUNNER_CACHE[n_tok]
    import jax
    from jax.sharding import Mesh, PartitionSpec, NamedSharding
    from jax.experimental.shard_map import shard_map
    from concourse.bass2jax import (
        _bass_exec_p, partition_id_tensor, install_neuronx_cc_hook)

    _install_legalizer()
    install_neuronx_cc_hook()
    nc = build_program(n_tok)

    partition_name = (nc.partition_id_tensor.name
                      if nc.partition_id_tensor else None)
    in_names, out_names, out_avals = [], [], []
    for alloc in nc.m.functions[0].allocations:
        if not isinstance(alloc, mybir.MemoryLocationSet):
            continue
        name = alloc.memorylocations[0].name
        if alloc.kind == "ExternalInput":
            if name != partition_name:
                in_names.append(name)
        elif alloc.kind == "ExternalOutput":
            out_names.append(name)
            out_avals.append(jax.core.ShapedArray(
                tuple(alloc.tensor_shape), mybir.dt.np(alloc.dtype)))
    all_in = list(in_names)
    if partition_name is not None:
        all_in.append(partition_name)

    def _body(*args):
        operands = list(args)
        if partition_name is not None:
            operands.append(partition_id_tensor())
        return tuple(_bass_exec_p.bind(
            *operands, out_avals=tuple(out_avals), in_names=tuple(all_in),
            out_names=tuple(out_names), lowering_input_output_aliases=(),
            sim_require_finite=True, sim_require_nnan=True, nc=nc))

    devices = jax.devices()[:8]
    mesh = Mesh(np.asarray(devices), ("core",))
    fn = jax.jit(shard_map(
        _body, mesh=mesh, in_specs=(PartitionSpec("core"),) * len(in_names),
        out_specs=(PartitionSpec("core"),) * len(out_names), check_rep=False),
        keep_unused=True)
    sharding = NamedSharding(mesh, PartitionSpec("core"))
    runner = (fn, in_names, out_names, sharding, jax)
    _RUNNER_CACHE[n_tok] = runner
    return runner


def _concat_inputs(x2d, Wq, Wk, Wv, Wo, bo, proj, n_tok):
    """Per-input global arrays, core-order concatenated on axis 0.
    Core c: batch c//4, token-quarter rank c%4, head-pair hp (0,1,2 global
    / 3 local). Static data (weights/tables, identical for cores c and
    c+4) ships as per-pair blob halves reassembled by a device AllGather:
    core c<4 sends rows [0:H), its pair partner rows [H:2H)."""
    NQ = n_tok // 4
    NCOS, R_SIN, R_END, R_TOT = _blob_rows(NQ)
    HALF = R_TOT // 2
    cos, sinN = _rotary_tables(n_tok)
    projT = np.ascontiguousarray(proj.T)
    blobs = []
    for m in range(4):
        blob = np.zeros((R_TOT, DM), np.float32)
        cs = slice(m * 128, (m + 1) * 128)
        # wq/wk/wv pre-rearranged to the SBUF layout [p, chunk, f]
        for r0, W in ((R_WQ, Wq), (R_WK, Wk), (R_WV, Wv)):
            blob[r0:r0 + 128] = (W[:, cs].reshape(4, 128, 128)
                                 .transpose(1, 0, 2).reshape(128, DM))
        blob[R_WO:R_WO + 128] = Wo[cs, :]
        blob[R_PROJ:R_PROJ + 32] = projT.reshape(32, DM)
        blob[R_BO] = bo * 0.25
        blob[R_COS:R_COS + NCOS] = cos[:, m * NQ:(m + 1) * NQ].reshape(-1, DM)
        blob[R_SIN:R_SIN + NCOS] = sinN[:, m * NQ:(m + 1) * NQ].reshape(-1, DM)
        blobs.append(blob)
    halves = [blobs[c % 4][0:HALF] if c < 4 else blobs[c % 4][HALF:R_TOT]
              for c in range(8)]
    return {"xq": x2d, "blob_h": np.concatenate(halves, 0)}


def kernel(x, Wq, Wk, Wv, Wo, bo, proj):
    x = np.ascontiguousarray(np.asarray(x, np.float32))
    Wq, Wk, Wv, Wo = (np.asarray(a, np.float32) for a in (Wq, Wk, Wv, Wo))
    bo = np.asarray(bo, np.float32)
    proj = np.asarray(proj, np.float32)
    B, n_tok, _ = x.shape
    assert B == 2 and n_tok % 2048 == 0

    fn, in_names, out_names, sharding, jax = _get_runner(n_tok)
    ins = _concat_inputs(x.reshape(B * n_tok, DM), Wq, Wk, Wv, Wo, bo, proj,
                         n_tok)
    dev = [jax.device_put(ins[name], sharding) for name in in_names]
    outs = fn(*dev)
    y = np.asarray(outs[out_names.index("y")])
    return y.reshape(B, n_tok, DM)


# revision 34
# speedup vs baseline: 1.1616x; 1.1616x over previous
"""DoubleAttention (Performer global heads + local windowed heads) on 8
Trainium2 NeuronCores via Bass/Tile SPMD.

I/O-lean sharding (v2). 8 cores = 2 batch-groups of 4: cores 0-3 own
batch 0, cores 4-7 batch 1. Within a group, cores rank r=0..3 each ship
ONE natural-layout quarter of x ([2048,512], a zero-copy row view), so
H2D for x is exactly |x|. On device each core transposes its quarter
(PE transposes) and the group AllGathers the feature-major x into DRAM.
Rotary cos/sin tables ship as per-core quarters and are AllGathered the
same way. Cores 0,1,2 (4,5,6) compute one pair of the 6 global
Performer heads over all 8192 tokens; cores 3,7 compute both local
windowed heads. Each core adds bo/4 (rank-1 matmul) to its partial
y = attn_out @ Wo[slice,:]; a ReduceScatter(add) over the group then
hands each core the FINAL y rows for its token quarter, so D2H is
exactly |y| and the host does no arithmetic at all.

Exact math restructurings vs the reference (fp-rounding-level equal):
 - Performer `ratio` cancels between numerator and denominator.
 - dd - diag fused into ONE matmul: contract [qT*norm ; qT^2*0.5norm^2]
   (128 rows) against [projT ; -ones].
 - per-query max only affects the eps floor: exp(z-m)+eps is a per-token
   scale (cancels) times exp(z) + eps*e^{m}; the latter enters num/den
   as an appended rank-1 term.
 - k-side global max applied post-hoc: ctx = e^{-m_k}*sum(exp(z_k)[v|1])
   + eps*[vsum|N].
 - local attention: softmax max-subtraction dropped (dots are O(5), exp
   safe in fp32; softmax is shift-invariant). P = exp(dots/8) computed
   KEY-major; [v|1] folds the row-sum into the same A@V matmul.

All heavy matmuls run as float32r (~2e-4 rel err, same order as this
PE's fp32 mode, at 4x throughput). Engines cannot shift partitions
(lane-locked), so the few cross-partition moves go through SBUF->SBUF
DMAs or PE transposes.
"""
import numpy as np
from contextlib import ExitStack

import concourse.bass as bass
import concourse.mybir as mybir
import concourse.tile as tile
from concourse.masks import make_identity

F32 = mybir.dt.float32
F32R = mybir.dt.float32r
BF16 = mybir.dt.bfloat16
AF = mybir.ActivationFunctionType
ALU = mybir.AluOpType
AX = mybir.AxisListType

DM = 512
DH = 64
NF = 256
WIN = 256
EPS = 1e-4
NORM = DH ** -0.25
SQRT_C2 = (0.5 * NORM * NORM) ** 0.5    # Square(x*s) = x^2 * 0.5norm^2
GROUPS = [[0, 1, 2, 3], [4, 5, 6, 7]]
PAIRS = [[0, 4], [1, 5], [2, 6], [3, 7]]

# static-input blob row offsets (512-f32-wide rows). Cores c and c+4 need
# IDENTICAL static data (weights by head-pair c%4, rotary by rank c%4), so
# each ships only half the blob and a pair AllGather reassembles it.
R_WQ, R_WK, R_WV, R_WO, R_PROJ, R_BO, R_COS = 0, 128, 256, 384, 512, 544, 545


def _blob_rows(NQ):
    ncos = DH * NQ // 512
    r_sin = R_COS + ncos
    r_end = r_sin + ncos
    r_tot = r_end + (r_end & 1)
    return ncos, r_sin, r_end, r_tot

# ---------------------------------------------------------------------------
# walrus wait legalizer: this toolchain's walrus accepts only ONE sync wait
# per instruction; Tile attaches several. Split extras onto NoOps.
# ---------------------------------------------------------------------------
_WNOP = [0]


def _nop(engine, debug, waits=(), updates=()):
    _WNOP[0] += 1
    return {
        "name": f"WNOP-{_WNOP[0]}",
        "opcode": "NoOp",
        "engine": engine,
        "ins": [],
        "outs": [],
        "debug": debug,
        "sync_info": {"on_update": list(updates), "on_wait": list(waits)},
    }


def _legalize_bir_waits(bir_bytes: bytes) -> bytes:
    """Engine instruction structs accept ONE wait and ONE sem-inc(+1)
    update. Tile emits several waits per instruction and (at If-arm clock
    merges) big sem-add-imm updates. Split extras onto NoOps on the same
    queue (engines retire in order, so ordering semantics hold)."""
    import orjson
    d = orjson.loads(bir_bytes)
    for fn in d["functions"]:
        for bb in fn["blocks"]:
            out = []
            for inst in bb["instructions"]:
                op = inst.get("opcode", "")
                si = inst.get("sync_info")
                if si is None or "Branch" in op:
                    out.append(inst)
                    continue
                dbg = inst.get("debug")
                eng = inst["engine"]
                if "DMA" not in op.upper() and op != "ISA":
                    # A big add-imm comes from If-arm clock alignment: v-1
                    # virtual ticks (guarding no data) + this instruction's
                    # own completion. Emit the padding as +1 NoOps BEFORE
                    # the instruction (a trailing NoOp would fire at issue,
                    # before the writes drain) and keep +1 on it.
                    ups = si.get("on_update") or []
                    new_ups = []
                    for u in ups:
                        if (u.get("sync_type") == "semaphore"
                                and u.get("update_mode") in ("sem-inc",
                                                             "sem-add-imm")
                                and int(u.get("update_value", 1)) > 1):
                            v = int(u["update_value"])
                            out.append(_nop(eng, dbg, updates=[
                                dict(u, update_mode="sem-add-imm",
                                     update_value=v - 1)]))
                            new_ups.append(dict(u, update_mode="sem-inc",
                                                update_value=1))
                        else:
                            new_ups.append(u)
                    si["on_update"] = new_ups
                waits = si.get("on_wait") or []
                if len(waits) > 1:
                    for w in waits[:-1]:
                        out.append(_nop(eng, dbg, waits=[w]))
                    si["on_wait"] = [waits[-1]]
                out.append(inst)
            bb["instructions"] = out
    return orjson.dumps(d)


def _install_legalizer():
    import concourse.bass2jax as b2j
    if getattr(b2j, "_wait_legalizer_installed", False):
        return
    orig = b2j.compile_bir_kernel

    def patched(ant_bir_str, *args, **kwargs):
        return orig(_legalize_bir_waits(ant_bir_str), *args, **kwargs)

    b2j.compile_bir_kernel = patched
    b2j._wait_legalizer_installed = True


# ---------------------------------------------------------------------------
# program builder
# ---------------------------------------------------------------------------

class _Env:
    pass


def build_program(n_tok: int) -> bass.Bass:
    TT = 512
    NST = n_tok // TT
    NC = n_tok // 128
    NW = n_tok // WIN
    NQ = n_tok // 4          # tokens per core quarter
    TPC = NQ // TT           # 512-token tiles per quarter/chunk

    nc = bass.Bass(num_devices=8)
    e = _Env()
    e.n_tok, e.TT, e.NST, e.NC, e.NW, e.NQ, e.TPC = (
        n_tok, TT, NST, NC, NW, NQ, TPC)
    NCOS, R_SIN, R_END, R_TOT = _blob_rows(NQ)
    e.xq = nc.declare_dram_parameter("xq", [NQ, DM], F32, isOutput=False)
    e.blob_h = nc.declare_dram_parameter("blob_h", [R_TOT // 2, DM], F32,
                                         isOutput=False)
    e.y = nc.declare_dram_parameter("y", [NQ, DM], F32, isOutput=True)

    with ExitStack() as ctx:
        tc = ctx.enter_context(tile.TileContext(nc))

        # ---- DRAM scratch (collective bounces + gathered views) ----
        # x gather and y reduce-scatter are split 4 ways and pipelined:
        # sub-gather s covers 512-token sub-blocks s of every rank quarter
        # (consumed s-major), output group j covers final rows
        # r*NQ + j*512.. of every rank r (produced j-major).
        dram = ctx.enter_context(tc.tile_pool(name="dram", bufs=1, space="DRAM"))
        e.xtq_b = dram.tile([4, 128, 4, TT], F32)    # my transposed quarter
        e.xg = dram.tile([4, 4, 128, 4, TT], F32)    # [s, rank, p, c, t]
        e.blob_b = dram.tile([R_TOT // 2, DM], F32)  # pair-AG bounce
        e.blob = dram.tile([R_TOT, DM], F32)         # pair-gathered statics
        e.cosg = dram.tile([4, DH, NQ], F32)
        e.sing = dram.tile([4, DH, NQ], F32)
        e.yacc = dram.tile([4, 4, TT, DM], F32)      # [j, rank, u, d] partials
        e.yrs = dram.tile([NQ, DM], F32)             # reduce-scattered y rows

        # ---- reassemble the pair-shared statics, then load them ----
        nc.gpsimd.dma_start(e.blob_b[:], e.blob_h[:])
        nc.gpsimd.collective_compute(
            "AllGather", ALU.bypass, replica_groups=PAIRS,
            ins=[e.blob_b.opt()], outs=[e.blob.opt()])

        # ---- shared preamble ----
        pre = ctx.enter_context(tc.tile_pool(name="pre", bufs=1))
        e.ident = pre.tile([128, 128], F32)
        make_identity(nc, e.ident[:])

        e.wq_r = pre.tile([128, 4, 128], F32R)
        e.wk_r = pre.tile([128, 4, 128], F32R)
        e.wv_r = pre.tile([128, 4, 128], F32R)
        for w_sb, r0 in ((e.wq_r, R_WQ), (e.wk_r, R_WK), (e.wv_r, R_WV)):
            nc.sync.dma_start(
                w_sb[:], e.blob[r0:r0 + 128, :].bitcast(F32R).rearrange(
                    "p (c f) -> p c f", c=4))
        e.wo_r = pre.tile([128, DM], F32R)
        nc.sync.dma_start(e.wo_r[:], e.blob[R_WO:R_WO + 128, :].bitcast(F32R))
        e.bo4_row = pre.tile([1, DM], F32)
        nc.sync.dma_start(e.bo4_row[:], e.blob[R_BO:R_BO + 1, :])
        e.ones_1x128 = pre.tile([1, 128], F32)
        nc.gpsimd.memset(e.ones_1x128[:], 1.0)

        pn_f = pre.tile([128, NF + 4], F32)
        nc.sync.dma_start(pn_f[0:DH, 0:NF],
                          e.blob[R_PROJ:R_PROJ + 32, :].rearrange(
                              "a (b c) -> (a b) c", b=2))
        nc.gpsimd.memset(pn_f[DH:128, 0:NF], -1.0)
        # col 256 extracts diag (sum of the squared half); 257-259 pad (f32r
        # moving free dim must be a multiple of 4)
        nc.gpsimd.memset(pn_f[0:DH, NF:NF + 4], 0.0)
        nc.gpsimd.memset(pn_f[DH:128, NF:NF + 4], 0.0)
        nc.gpsimd.memset(pn_f[DH:128, NF:NF + 1], 1.0)
        e.projnegP_r = pre.tile([128, NF + 4], F32R)
        nc.vector.tensor_copy(e.projnegP_r[:], pn_f[:])
        e.projneg_r = e.projnegP_r[:, 0:NF]

        ones_f = pre.tile([128, 1], F32)
        nc.gpsimd.memset(ones_f[:], 1.0)
        e.ones_f = ones_f
        e.ones_col_r = pre.tile([128, 1], F32R)
        nc.vector.tensor_copy(e.ones_col_r[:], ones_f[:])
        e.ones_row65 = pre.tile([1, 65], F32)
        nc.gpsimd.memset(e.ones_row65[:], 1.0)
        e.lneps = pre.tile([128, 1], F32)
        nc.gpsimd.memset(e.lneps[:], float(np.log(EPS)))

        # ---- shared residents (used by BOTH branches; SBUF is core-local) ----
        res = ctx.enter_context(tc.tile_pool(name="res", bufs=1))
        e.R1 = res.tile([128, n_tok], F32R)        # global: Aq head0 / local: qTr
        e.R2 = res.tile([128, n_tok], F32R)        # global: Aq head1 / local: kTr
        e.R3 = res.tile([128, NC, 130], F32R)      # v token-major [v0|1|v1|1]
        e.mk_buf = res.tile([128, 2, NC], F32)
        e.vsum_buf = res.tile([128, max(NST, 2)], F32)
        e.ctx_fm = res.tile([128, 2, 2, 68], F32R)  # [p, mchunk, head, col] (68: f32r moving needs %4)
        e.s_row = res.tile([1, 2, 68], F32R)

        # init the ones columns of R3 once (cols 64 and 129 of each chunk)
        for kc in range(NC):
            nc.vector.tensor_copy(e.R3[:, kc, 64:65], e.ones_col_r[:])
            nc.vector.tensor_copy(e.R3[:, kc, 129:130], e.ones_col_r[:])

        # ---- transpose own x quarter and gather the group's xT ----
        # scoped pools: this phase's SBUF/PSUM is released before the
        # branch working pools open (SBUF is within 8KB of full).
        with tc.tile_pool(name="xp", bufs=2) as xp, \
                tc.tile_pool(name="psX", bufs=2, space="PSUM") as psX:
            for s in range(4):
                for ii in range(4):
                    i = s * 4 + ii
                    xn = xp.tile([128, DM], F32, tag="xn")
                    nc.sync.dma_start(xn[:], e.xq[i * 128:(i + 1) * 128, :])
                    xtq = xp.tile([128, 4, 128], F32, tag="xtq")
                    for c in range(4):
                        tr_ps = psX.tile([128, 128], F32, tag="smx")
                        nc.tensor.transpose(tr_ps[:],
                                            xn[:, c * 128:(c + 1) * 128],
                                            e.ident[:])
                        nc.vector.tensor_copy(xtq[:, c, :], tr_ps[:])
                    nc.sync.dma_start(
                        e.xtq_b[s, :, :, ii * 128:(ii + 1) * 128], xtq[:])
                nc.gpsimd.collective_compute(
                    "AllGather", ALU.bypass, replica_groups=GROUPS,
                    ins=[e.xtq_b[s].opt()], outs=[e.xg[s].opt()])
                if s == 0:
                    # rotary tables aren't consumed until after x sub-
                    # gather 0 lands, so gather them BEHIND it: keeps the
                    # collective queue clear ahead of the compute-gating
                    # AG_0 (global cores' critical path).
                    nc.gpsimd.collective_compute(
                        "AllGather", ALU.bypass, replica_groups=GROUPS,
                        ins=[e.blob[R_COS:R_COS + NCOS, :].opt()],
                        outs=[e.cosg.opt()])
                    nc.gpsimd.collective_compute(
                        "AllGather", ALU.bypass, replica_groups=GROUPS,
                        ins=[e.blob[R_SIN:R_SIN + NCOS, :].opt()],
                        outs=[e.sing.opt()])

        # ---- shared pools (tags shared across branches to bound SBUF) ----
        e.ld = ctx.enter_context(tc.tile_pool(name="ld", bufs=2))
        e.wk3 = ctx.enter_context(tc.tile_pool(name="wk3", bufs=3))
        e.wk2 = ctx.enter_context(tc.tile_pool(name="wk2", bufs=2))
        e.psProj = ctx.enter_context(tc.tile_pool(name="psProj", bufs=3, space="PSUM"))
        e.psAcc = ctx.enter_context(tc.tile_pool(name="psAcc", bufs=2, space="PSUM"))
        e.psSm = ctx.enter_context(tc.tile_pool(name="psSm", bufs=3, space="PSUM"))

        pid = nc.partition_id()
        is_global = (pid & 3) < 3
        with tc.If(is_global) as cmp:
            _global_phase1(nc, tc, e)
        with cmp.Else():
            _local_phase1(nc, tc, e)

        # ---- output groups: compute j while reduce-scattering j-1 ----
        for j in range(4):
            with tc.If(is_global) as cmpj:
                _global_out_j(nc, tc, e, j)
            with cmpj.Else():
                _local_out_j(nc, tc, e, j)
            nc.gpsimd.collective_compute(
                "ReduceScatter", ALU.add, replica_groups=GROUPS,
                ins=[e.yacc[j].opt()], outs=[e.yrs[j * TT:(j + 1) * TT, :].opt()])
            nc.gpsimd.dma_start(e.y[j * TT:(j + 1) * TT, :],
                                e.yrs[j * TT:(j + 1) * TT, :])

    return nc


def _tr(nc, e, out_ap, in_ap):
    k = in_ap.shape[0]
    nc.tensor.transpose(out_ap, in_ap, e.ident[0:k, 0:k])


def _load_xt(nc, e, t):
    s, r = t % e.TPC, t // e.TPC
    xt = e.ld.tile([128, 4, e.TT], F32R, tag="xt")
    nc.sync.dma_start(xt[:], e.xg[s, r].bitcast(F32R))
    return xt


def _tile_order(e):
    """s-major: tiles using sub-gather s come before any using s+1, so
    compute on s overlaps the AllGather of s+1."""
    return [r * e.TPC + s for s in range(e.TPC) for r in range(4)]


def _project(nc, e, xt, w_r):
    """q/k/v projection into PSUM [128 = 2 heads x 64, TT]."""
    pp = e.psProj.tile([128, e.TT], F32, tag="proj")
    for c in range(4):
        nc.tensor.matmul(pp[:], w_r[:, c, :], xt[:, c, :],
                         start=(c == 0), stop=(c == 3))
    return pp


def _v_tokmajor(nc, e, t, v_ps):
    """v [128, TT] PSUM d-major -> R3 chunks [tok128, v0|1|v1|1]."""
    v_sb = e.wk2.tile([128, e.TT], F32, tag="vsb")
    nc.scalar.activation(v_sb[:], v_ps[:], AF.Identity,
                         accum_out=e.vsum_buf[:, t:t + 1])
    for su in range(4):
        kc = t * 4 + su
        vtr_ps = e.psSm.tile([128, 128], F32, tag="sm")
        _tr(nc, e, vtr_ps[:], v_sb[:, su * 128:(su + 1) * 128])
        nc.vector.tensor_copy(e.R3[:, kc, 0:64], vtr_ps[:, 0:64])
        nc.vector.tensor_copy(e.R3[:, kc, 65:129], vtr_ps[:, 64:128])


def _aug_assemble(nc, e, p_ps, dest0, dest1, ts, tag):
    """[2-head packed PSUM [128,TT]] -> per-head augmented [n*NORM ; n^2*c2]
    written into dest0/dest1 [128, ts]. Lane engines can't shift partitions,
    so the cross-half moves are SBUF->SBUF DMAs."""
    qn = e.wk2.tile([128, e.TT], F32R, tag=f"{tag}n")
    nc.scalar.mul(qn[:], p_ps[:], NORM)
    sq = e.wk2.tile([128, e.TT], F32R, tag=f"{tag}s")
    nc.scalar.activation(sq[:], p_ps[:], AF.Square, scale=SQRT_C2)
    nc.vector.tensor_copy(dest0[0:64, ts], qn[0:64, :])
    nc.sync.dma_start(dest0[64:128, ts], sq[0:64, :])
    nc.sync.dma_start(dest1[0:64, ts], qn[64:128, :])
    nc.vector.tensor_copy(dest1[64:128, ts], sq[64:128, :])


def _global_phase1(nc, tc, e):
    NST, NC, TT, n_tok = e.NST, e.NC, e.TT, e.n_tok
    Aq = [e.R1, e.R2]

    # ---------------- phase G1: k/v side + Aq build ----------------
    ctx_ps = []
    for h in range(2):
        acc_t = e.psAcc.tile([65, NF], F32, tag="acc", name=f"acc{h}")
        ctx_ps.append(acc_t)
    order = _tile_order(e)
    for it, t in enumerate(order):
        ts = slice(t * TT, (t + 1) * TT)
        xt = _load_xt(nc, e, t)

        q_ps = _project(nc, e, xt, e.wq_r)
        _aug_assemble(nc, e, q_ps, Aq[0], Aq[1], ts, "q")

        k_ps = _project(nc, e, xt, e.wk_r)
        ak0 = e.wk2.tile([128, TT], F32R, tag="ak0")
        ak1 = e.wk2.tile([128, TT], F32R, tag="ak1")
        _aug_assemble(nc, e, k_ps, ak0, ak1, slice(0, TT), "k")
        ak = [ak0, ak1]

        v_ps = _project(nc, e, xt, e.wv_r)
        _v_tokmajor(nc, e, t, v_ps)

        for su in range(4):
            kc = t * 4 + su
            ss = slice(su * 128, (su + 1) * 128)
            for h in range(2):
                zk_ps = e.psSm.tile([128, NF + 4], F32, tag="sm")
                nc.tensor.matmul(zk_ps[:], ak[h][:, ss], e.projnegP_r[:],
                                 start=True, stop=True)
                # reference maxes are over dd = z + diag (diag in col 256)
                zmax = e.wk3.tile([128, 1], F32, tag="zmax")
                nc.vector.reduce_max(zmax[:], zk_ps[:, 0:NF], axis=AX.X)
                nc.vector.tensor_tensor(e.mk_buf[:, h, kc:kc + 1], zmax[:],
                                        zk_ps[:, NF:NF + 1], ALU.add)
                kp = e.wk3.tile([128, NF], F32R, tag="kp")
                nc.scalar.activation(kp[:], zk_ps[:, 0:NF], AF.Exp)
                nc.tensor.matmul(ctx_ps[h][:], e.R3[:, kc, h * 65:(h + 1) * 65],
                                 kp[:], start=(it == 0 and su == 0),
                                 stop=(it == NST - 1 and su == 3))

    # ---- k-side fixups ----
    vsum = e.wk2.tile([128, 1], F32, tag="vsum")
    nc.vector.reduce_sum(vsum[:], e.vsum_buf[:, 0:NST], axis=AX.X)
    vst_ps = e.psSm.tile([1, 128], F32, tag="sm")
    _tr(nc, e, vst_ps[:], vsum[:])
    vsumT = e.wk2.tile([1, 128], F32, tag="vsumT", bufs=1)
    nc.vector.tensor_copy(vsumT[:], vst_ps[:])

    for h in range(2):
        mk_red = e.wk2.tile([128, 1], F32, tag="mkred")
        nc.vector.reduce_max(mk_red[:], e.mk_buf[:, h, :], axis=AX.X)
        mkt_ps = e.psSm.tile([1, 128], F32, tag="sm")
        _tr(nc, e, mkt_ps[:], mk_red[:])
        mkt = e.wk2.tile([1, 128], F32, tag="mkt", bufs=1)
        nc.vector.tensor_copy(mkt[:], mkt_ps[:])
        mk_sc = e.wk2.tile([1, 1], F32, tag="mksc")
        nc.vector.reduce_max(mk_sc[:], mkt[:], axis=AX.X)
        f_sc = e.wk2.tile([1, 1], F32, tag="fsc")
        nc.scalar.activation(f_sc[:], mk_sc[:], AF.Exp, scale=-1.0)
        fb_ps = e.psSm.tile([65, 1], F32, tag="sm")
        nc.tensor.matmul(fb_ps[:], e.ones_row65[:], f_sc[:], start=True, stop=True)
        fb = e.wk2.tile([65, 1], F32, tag="fb")
        nc.vector.tensor_copy(fb[:], fb_ps[:])

        ev_row = e.wk2.tile([1, 65], F32, tag="evrow", bufs=1)
        nc.scalar.mul(ev_row[:, 0:64], vsumT[:, h * DH:(h + 1) * DH], EPS)
        nc.gpsimd.memset(ev_row[:, 64:65], EPS * n_tok)
        ev_ps = e.psSm.tile([65, 1], F32, tag="sm")
        _tr(nc, e, ev_ps[:], ev_row[:])
        epsv = e.wk2.tile([65, 1], F32, tag="epsv", bufs=1)
        nc.vector.tensor_copy(epsv[:], ev_ps[:])

        ctxT = e.wk2.tile([65, NF], F32, tag="ctxT", bufs=1)
        nc.vector.tensor_scalar(ctxT[:], ctx_ps[h][:], fb[:], epsv[:],
                                ALU.mult, ALU.add)
        for c in range(2):
            cf_ps = e.psSm.tile([128, 65], F32, tag="sm")
            _tr(nc, e, cf_ps[:], ctxT[:, c * 128:(c + 1) * 128])
            nc.vector.tensor_copy(e.ctx_fm[:, c, h, 0:65], cf_ps[:])
            nc.vector.tensor_copy(e.ctx_fm[:, c, h, 65:68], cf_ps[:, 0:3])
        sr_ps = e.psSm.tile([1, 65], F32, tag="sm")
        for c in range(2):
            nc.tensor.matmul(sr_ps[:], e.ones_f[:],
                             e.ctx_fm[:, c, h, 0:65].bitcast(F32),
                             start=(c == 0), stop=(c == 1))
        nc.vector.tensor_copy(e.s_row[:, h, 0:65], sr_ps[:])
        nc.vector.tensor_copy(e.s_row[:, h, 65:68], sr_ps[:, 0:3])


def _global_out_j(nc, tc, e, j):
    # ---------------- phase G2: q side, output group j ----------------
    NST, TT = e.NST, e.TT
    Aq = [e.R1, e.R2]
    for r in range(4):
        t = r * e.TPC + j
        ts = slice(t * TT, (t + 1) * TT)
        qp = [[None, None], [None, None]]
        ert = [None, None]
        for h in range(2):
            for c in range(2):
                zf_ps = e.psProj.tile([128, TT], F32, tag="proj")
                nc.tensor.matmul(zf_ps[:], e.projneg_r[:, c * 128:(c + 1) * 128],
                                 Aq[h][:, ts], start=True, stop=True)
                qp_c = e.wk2.tile([128, TT], F32R, tag=f"qp{h}{c}")
                nc.scalar.activation(qp_c[:], zf_ps[:], AF.Exp)
                qp[h][c] = qp_c
            ert_h = []
            for su in range(4):
                zt_ps = e.psSm.tile([128, NF + 4], F32, tag="sm")
                nc.tensor.matmul(
                    zt_ps[:], Aq[h][:, t * TT + su * 128: t * TT + (su + 1) * 128],
                    e.projnegP_r[:], start=True, stop=True)
                zmax = e.wk3.tile([128, 1], F32, tag="zmax")
                nc.vector.reduce_max(zmax[:], zt_ps[:, 0:NF], axis=AX.X)
                mq = e.wk3.tile([128, 1], F32, tag="mq")
                nc.vector.tensor_tensor(mq[:], zmax[:], zt_ps[:, NF:NF + 1], ALU.add)
                er = e.wk3.tile([128, 1], F32, tag="er")
                nc.scalar.activation(er[:], mq[:], AF.Exp, bias=e.lneps[:])
                ert_ps = e.psSm.tile([1, 128], F32, tag="sm")
                _tr(nc, e, ert_ps[:], er[:])
                ert_su = e.wk3.tile([1, 128], F32R, tag=f"ert{h}", name=f"ert{h}_{su}")
                nc.vector.tensor_copy(ert_su[:], ert_ps[:])
                ert_h.append(ert_su)
            ert[h] = ert_h

        for su in range(4):
            ss = slice(su * 128, (su + 1) * 128)
            row0 = t * TT + su * 128
            ao = e.wk3.tile([128, 128], F32, tag="ao")
            for h in range(2):
                nd_ps = e.psSm.tile([128, 68], F32, tag="sm")
                nc.tensor.matmul(nd_ps[:], qp[h][0][:, ss], e.ctx_fm[:, 0, h, :],
                                 start=True, stop=False)
                nc.tensor.matmul(nd_ps[:], qp[h][1][:, ss], e.ctx_fm[:, 1, h, :],
                                 start=False, stop=False)
                nc.tensor.matmul(nd_ps[:], ert[h][su][:], e.s_row[:, h, :],
                                 start=False, stop=True)
                rec = e.wk3.tile([128, 1], F32, tag="rec")
                nc.vector.reciprocal(rec[:], nd_ps[:, 64:65])
                nc.vector.tensor_scalar_mul(ao[:, h * DH:(h + 1) * DH],
                                            nd_ps[:, 0:64], rec[:])
            _project_out(nc, e, ao, row0)


def _project_out(nc, e, ao, row0):
    """attn-out token-major [128,128] -> transpose -> yacc rows via Wo slice
    (+ bo/4 as a rank-1 matmul; the 4 group partials sum to + bo)."""
    aoT_ps = e.psSm.tile([128, 128], F32, tag="sm")
    _tr(nc, e, aoT_ps[:], ao[:])
    aoT = e.wk3.tile([128, 128], F32R, tag="aoTs")
    nc.vector.tensor_copy(aoT[:], aoT_ps[:])
    y_ps = e.psProj.tile([128, DM], F32, tag="proj")
    nc.tensor.matmul(y_ps[:], aoT[:], e.wo_r[:], start=True, stop=False)
    nc.tensor.matmul(y_ps[:], e.ones_1x128[:], e.bo4_row[:],
                     start=False, stop=True)
    y_sb = e.wk2.tile([128, DM], F32, tag="ysb")
    nc.scalar.copy(y_sb[:], y_ps[:])
    r, rem = divmod(row0, e.NQ)
    j, u = divmod(rem, e.TT)
    nc.sync.dma_start(e.yacc[j, r, u:u + 128, :], y_sb[:])


def _local_phase1(nc, tc, e):
    NST, NC, TT, NW, TPC = e.NST, e.NC, e.TT, e.NW, e.TPC
    qTr, kTr = e.R1, e.R2

    # ---------------- phase L1: projections + rotary ----------------
    for t in _tile_order(e):
        ts = slice(t * TT, (t + 1) * TT)
        chunk, off = t // TPC, (t % TPC) * TT
        xt = _load_xt(nc, e, t)
        cos2 = e.ld.tile([128, TT], F32, tag="cos2")
        nc.sync.dma_start(cos2[0:DH, :], e.cosg[chunk, :, off:off + TT])
        nc.sync.dma_start(cos2[DH:128, :], cos2[0:DH, :])
        sin2 = e.ld.tile([128, TT], F32, tag="sin2")
        nc.sync.dma_start(sin2[0:DH, :], e.sing[chunk, :, off:off + TT])
        nc.sync.dma_start(sin2[DH:128, :], sin2[0:DH, :])

        for w_r, dest in ((e.wq_r, qTr), (e.wk_r, kTr)):
            pp = _project(nc, e, xt, w_r)
            p_sb = e.wk2.tile([128, TT], F32, tag="qn")
            nc.vector.tensor_copy(p_sb[:], pp[:])
            p_sw = e.wk2.tile([128, TT], F32, tag="qs")
            for h in range(2):
                o = h * DH
                nc.sync.dma_start(p_sw[o:o + 32, :], p_sb[o + 32:o + 64, :])
                nc.sync.dma_start(p_sw[o + 32:o + 64, :], p_sb[o:o + 32, :])
            t1 = e.wk2.tile([128, TT], F32, tag="kn")
            nc.vector.tensor_tensor(t1[:], p_sb[:], cos2[:], ALU.mult)
            t2 = e.wk2.tile([128, TT], F32, tag="ks")
            nc.vector.tensor_tensor(t2[:], p_sw[:], sin2[:], ALU.mult)
            nc.vector.tensor_tensor(dest[:, ts], t1[:], t2[:], ALU.add)

        v_ps = _project(nc, e, xt, e.wv_r)
        _v_tokmajor(nc, e, t, v_ps)


def _local_out_j(nc, tc, e, j):
    # ---------------- phase L2: windowed attention, output group j ----
    # Windows processed in PAIRS: adjacent windows share k-chunks, so one
    # [128, 512] dots matmul + one exp covers both windows per k-chunk
    # (halves the ACT op count and the dots matmul count).
    NC, NW, WINp = e.NC, e.NW, WIN
    qTr, kTr = e.R1, e.R2
    for r in range(4):
        wp = r * e.TPC + j
        wA, wB = 2 * wp, 2 * wp + 1
        qs = slice(wA * WIN, (wB + 1) * WIN)          # 512 queries
        cA0, cA1 = max(0, 2 * wA - 2), min(NC - 1, 2 * wA + 3)
        cB0, cB1 = max(0, 2 * wB - 2), min(NC - 1, 2 * wB + 3)
        olT_all = {}
        for h in range(2):
            hs = slice(h * DH, (h + 1) * DH)
            olA = e.psAcc.tile([65, WIN], F32, tag="acc", name=f"olA_{h}")
            olB = e.psAcc.tile([65, WIN], F32, tag="acc", name=f"olB_{h}")
            for kc in range(cA0, cB1 + 1):
                dk_ps = e.psProj.tile([128, 2 * WIN], F32, tag="proj")
                nc.tensor.matmul(dk_ps[:], kTr[hs, kc * 128:(kc + 1) * 128],
                                 qTr[hs, qs], start=True, stop=True)
                P = e.wk3.tile([128, 2 * WIN], F32R, tag="P2")
                nc.scalar.activation(P[:], dk_ps[:], AF.Exp, scale=0.125)
                if cA0 <= kc <= cA1:
                    nc.tensor.matmul(olA[:], e.R3[:, kc, h * 65:(h + 1) * 65],
                                     P[:, 0:WIN], start=(kc == cA0),
                                     stop=(kc == cA1))
                if cB0 <= kc <= cB1:
                    nc.tensor.matmul(olB[:], e.R3[:, kc, h * 65:(h + 1) * 65],
                                     P[:, WIN:2 * WIN], start=(kc == cB0),
                                     stop=(kc == cB1))
            for w, olp in ((wA, olA), (wB, olB)):
                olT_h = e.wk3.tile([65, WIN], F32, tag="olT", bufs=4,
                                   name=f"olT{w}_{h}")
                nc.vector.tensor_copy(olT_h[:], olp[:])
                olT_all[(h, w)] = olT_h
        for w in (wA, wB):
            olT = [olT_all[(0, w)], olT_all[(1, w)]]
            for su in range(2):
                row0 = w * WIN + su * 128
                ao = e.wk3.tile([128, 128], F32, tag="ao")
                for h in range(2):
                    tr_ps = e.psSm.tile([128, 65], F32, tag="sm")
                    _tr(nc, e, tr_ps[:], olT[h][:, su * 128:(su + 1) * 128])
                    rec = e.wk3.tile([128, 1], F32, tag="rec")
                    nc.vector.reciprocal(rec[:], tr_ps[:, 64:65])
                    nc.vector.tensor_scalar_mul(ao[:, h * DH:(h + 1) * DH],
                                                tr_ps[:, 0:64], rec[:])
                _project_out(nc, e, ao, row0)


# ---------------------------------------------------------------------------
# host wrapper: cached jitted shard_map runner, minimal copies
# ---------------------------------------------------------------------------
_RUNNER_CACHE = {}
_TABLE_CACHE = {}


def _rotary_tables(n_tok: int):
    if n_tok not in _TABLE_CACHE:
        inv_freq = 1.0 / (10000.0 ** (np.arange(0, DH, 2, dtype=np.float32) / DH))
        t = np.arange(n_tok, dtype=np.float32)
        freqs = t[:, None] * inv_freq[None, :]
        freqs = np.concatenate([freqs, freqs], axis=-1)
        cos = np.ascontiguousarray(np.cos(freqs).T.astype(np.float32))
        sin = np.sin(freqs).T.astype(np.float32)
        sinN = np.ascontiguousarray(
            np.concatenate([-sin[0:32], sin[32:64]], axis=0))
        _TABLE_CACHE[n_tok] = (cos, sinN)
    return _TABLE_CACHE[n_tok]


def _get_runner(n_tok: int):
    if n_tok in _RUNNER_CACHE:
        return _RUNNER_CACHE[n_tok]
    import jax
    from jax.sharding import Mesh, PartitionSpec, NamedSharding
    from jax.experimental.shard_map import shard_map
    from concourse.bass2jax import (
        _bass_exec_p, partition_id_tensor, install_neuronx_cc_hook)

    _install_legalizer()
    install_neuronx_cc_hook()
    nc = build_program(n_tok)

    partition_name = (nc.partition_id_tensor.name
                      if nc.partition_id_tensor else None)
    in_names, out_names, out_avals = [], [], []
    for alloc in nc.m.functions[0].allocations:
        if not isinstance(alloc, mybir.MemoryLocationSet):
            continue
        name = alloc.memorylocations[0].name
        if alloc.kind == "ExternalInput":
            if name != partition_name:
                in_names.append(name)
        elif alloc.kind == "ExternalOutput":
            out_names.append(name)
            out_avals.append(jax.core.ShapedArray(
                tuple(alloc.tensor_shape), mybir.dt.np(alloc.dtype)))
    all_in = list(in_names)
    if partition_name is not None:
        all_in.append(partition_name)

    def _body(*args):
        operands = list(args)
        if partition_name is not None:
            operands.append(partition_id_tensor())
        return tuple(_bass_exec_p.bind(
            *operands, out_avals=tuple(out_avals), in_names=tuple(all_in),
            out_names=tuple(out_names), lowering_input_output_aliases=(),
            sim_require_finite=True, sim_require_nnan=True, nc=nc))

    devices = jax.devices()[:8]
    mesh = Mesh(np.asarray(devices), ("core",))
    fn = jax.jit(shard_map(
        _body, mesh=mesh, in_specs=(PartitionSpec("core"),) * len(in_names),
        out_specs=(PartitionSpec("core"),) * len(out_names), check_rep=False),
        keep_unused=True)
    sharding = NamedSharding(mesh, PartitionSpec("core"))
    runner = (fn, in_names, out_names, sharding, jax)
    _RUNNER_CACHE[n_tok] = runner
    return runner


def _concat_inputs(x2d, Wq, Wk, Wv, Wo, bo, proj, n_tok):
    """Per-input global arrays, core-order concatenated on axis 0.
    Core c: batch c//4, token-quarter rank c%4, head-pair hp (0,1,2 global
    / 3 local). Static data (weights/tables, identical for cores c and
    c+4) ships as per-pair blob halves reassembled by a device AllGather:
    core c<4 sends rows [0:H), its pair partner rows [H:2H)."""
    NQ = n_tok // 4
    NCOS, R_SIN, R_END, R_TOT = _blob_rows(NQ)
    HALF = R_TOT // 2
    cos, sinN = _rotary_tables(n_tok)
    projT = np.ascontiguousarray(proj.T)
    blobs = []
    for m in range(4):
        blob = np.zeros((R_TOT, DM), np.float32)
        cs = slice(m * 128, (m + 1) * 128)
        # wq/wk/wv pre-rearranged to the SBUF layout [p, chunk, f]
        for r0, W in ((R_WQ, Wq), (R_WK, Wk), (R_WV, Wv)):
            blob[r0:r0 + 128] = (W[:, cs].reshape(4, 128, 128)
                                 .transpose(1, 0, 2).reshape(128, DM))
        blob[R_WO:R_WO + 128] = Wo[cs, :]
        blob[R_PROJ:R_PROJ + 32] = projT.reshape(32, DM)
        blob[R_BO] = bo * 0.25
        blob[R_COS:R_COS + NCOS] = cos[:, m * NQ:(m + 1) * NQ].reshape(-1, DM)
        blob[R_SIN:R_SIN + NCOS] = sinN[:, m * NQ:(m + 1) * NQ].reshape(-1, DM)
        blobs.append(blob)
    halves = [blobs[c % 4][0:HALF] if c < 4 else blobs[c % 4][HALF:R_TOT]
              for c in range(8)]
    return {"xq": x2d, "blob_h": np.concatenate(halves, 0)}


def kernel(x, Wq, Wk, Wv, Wo, bo, proj):
    x = np.ascontiguousarray(np.asarray(x, np.float32))
    Wq, Wk, Wv, Wo = (np.asarray(a, np.float32) for a in (Wq, Wk, Wv, Wo))
    bo = np.asarray(bo, np.float32)
    proj = np.asarray(proj, np.float32)
    B, n_tok, _ = x.shape
    assert B == 2 and n_tok % 2048 == 0

    fn, in_names, out_names, sharding, jax = _get_runner(n_tok)
    ins = _concat_inputs(x.reshape(B * n_tok, DM), Wq, Wk, Wv, Wo, bo, proj,
                         n_tok)
    dev = [jax.device_put(ins[name], sharding) for name in in_names]
    outs = fn(*dev)
    y = np.asarray(outs[out_names.index("y")])
    return y.reshape(B, n_tok, DM)
